# revision 1
# baseline (speedup 1.0000x reference)
"""RNN-T joint network kernel for 8 Trainium2 NeuronCores.

out[b,t,u,:] = W_out @ tanh(W_enc @ enc[b,t] + b_enc + W_dec @ dec[b,u]) + b_out

Sharding: data-parallel over B (8 batches -> 8 cores), weights replicated.

Two structural tricks vs the bf16 baseline (which is TensorE-bound at
~267us of bf16 streaming):

1. fp8-e4m3 DoubleRow for the big matmul (2 MACs/cell/cycle, K=256/mm).
   Direct fp8 quantization of tanh costs 3.6e-2 rel err, so the device
   quantizes the nonlinear residual g = tanh(x) - alpha*x (5x smaller
   rms -> 7.5e-3 total), and the exact linear part
   alpha*(W@e (+) W@d) + b_out (rank-structured) is added on the host,
   like the baseline's host-side b_out add.

2. The broadcast add x[j, t,u] = e[j,t] + d[j,u] is computed on the
   *TensorE* as a one-hot-indicator matmul instead of a DVE
   tensor_tensor (DVE was the second bottleneck): stationary is a
   stitched [d_projT (+b_enc) ; e_projT-chunk] tile [128, 128j], moving
   is a reusable 0/1 indicator [128, nt*U] whose column (t,u) has ones
   at rows u and 100+t.  pre lands in PSUM; ACT reads PSUM for tanh
   (faster than SBUF); DVE's only elementwise op is the fp8 STT.

Scale plumbing (free, folded into instruction scale fields):
  comb = -64a*(proj+bias) fp16; pre(psum) = -64a*x; t1 = tanh(pre*(-1/64a));
  g64 = t1*64 + pre = 64*(tanh(x)-a*x) fp8;  W fp8 = W.T*2048;
  out fp16 = psum * 2^-17.
"""

import numpy as np

B, T, U = 8, 200, 100
D = 512      # d_enc == d_dec
J = 512      # joint dim
V = 1024     # vocab
TU = T * U
NJ = J // 128
ND = D // 128

ALPHA = 0.7046830370401673
SG = 64.0                 # g scale into fp8
SW = 2048.0               # W_out scale into fp8
S = ALPHA * SG            # projection prescale magnitude
OUT_SCALE = 1.0 / (SG * SW)

GRP = 4                   # full tu-tiles staged per output DMA (1 MB)
DVE_COPY_FRAC = 0.45      # fraction of psum->sbuf copies on DVE
NT = 28                   # t values per chunk (K = 100+28 = 128 exact)
PSL = 1024                # pre-psum slice width (2 banks)

_CACHE = {}


def _chunks():
    sizes = [NT] * (T // NT) + ([T % NT] if T % NT else [])
    out, t0 = [], 0
    for s in sizes:
        out.append((t0, s))
        t0 += s
    return out


def _build():
    import concourse.bass as bass
    import concourse.mybir as mybir
    from concourse import tile

    f32 = mybir.dt.float32
    f16 = mybir.dt.float16
    f8 = mybir.dt.float8e4
    AF = mybir.ActivationFunctionType
    ALU = mybir.AluOpType
    DR = mybir.MatmulPerfMode.DoubleRow

    nc = bass.Bass()

    encT_d = nc.dram_tensor("encT", [D, T], f32, kind="ExternalInput")
    decT_d = nc.dram_tensor("decT", [D, U], f32, kind="ExternalInput")
    wencT_d = nc.dram_tensor("wencT", [D, J], f32, kind="ExternalInput")
    wdecT_d = nc.dram_tensor("wdecT", [D, J], f32, kind="ExternalInput")
    bencr_d = nc.dram_tensor("bencr", [1, J], f32, kind="ExternalInput")
    ones_d = nc.dram_tensor("ones", [1, 128], f32, kind="ExternalInput")
    ind_d = nc.dram_tensor("ind", [128, NT * U], f16, kind="ExternalInput")
    wq_d = nc.dram_tensor("wq", [2, 128, 2, V], f8, kind="ExternalInput")
    out_d = nc.dram_tensor("out", [TU, V], f16, kind="ExternalOutput")

    chunks = _chunks()
    NCH = len(chunks)

    with tile.TileContext(nc) as tc:
        with (
            tc.tile_pool(name="const", bufs=1) as cpool,
            tc.tile_pool(name="t1", bufs=3) as t1pool,
            tc.tile_pool(name="g", bufs=2) as gpool,
            tc.tile_pool(name="stage", bufs=4) as stpool,
            tc.tile_pool(name="stp", bufs=2) as stppool,
            tc.tile_pool(name="est", bufs=2) as estpool,
            tc.tile_pool(name="ps", bufs=4, space="PSUM") as pspool,
        ):
            # ---- constant loads -------------------------------------------
            wenc_sb = [cpool.tile([128, J], f32, tag=f"wenc{d}", name=f"wenc{d}") for d in range(ND)]
            wdec_sb = [cpool.tile([128, J], f32, tag=f"wdec{d}", name=f"wdec{d}") for d in range(ND)]
            enc_sb = [cpool.tile([128, T], f32, tag=f"enc{d}", name=f"enc{d}") for d in range(ND)]
            dec_sb = [cpool.tile([128, U], f32, tag=f"dec{d}", name=f"dec{d}") for d in range(ND)]
            bencr_sb = cpool.tile([1, J], f32, tag="bencr", name="bencr")
            ones_sb = cpool.tile([1, 128], f32, tag="ones", name="ones")
            ind_sb = cpool.tile([128, NT * U], f16, tag="ind", name="ind")
            wq_sb = [cpool.tile([128, 2, V], f8, tag=f"wq{jh}", name=f"wq{jh}") for jh in range(2)]
            # comb stationaries: one [128,128] slice per (chunk, j-tile);
            # rows 0:100 = -64a*(dpT + b_enc), rows 100:100+nt = -64a*epT
            comb = cpool.tile([128, NCH * NJ * 128], f16, tag="comb", name="comb")
            dpTb = cpool.tile([128, J], f16, tag="dpTb", name="dpTb")

            for d in range(ND):
                sl = slice(d * 128, (d + 1) * 128)
                nc.sync.dma_start(wdec_sb[d][:], wdecT_d[sl, :])
                nc.sync.dma_start(dec_sb[d][:], decT_d[sl, :])
            nc.sync.dma_start(bencr_sb[:], bencr_d[:])
            nc.sync.dma_start(ones_sb[:], ones_d[:])
            for d in range(ND):
                sl = slice(d * 128, (d + 1) * 128)
                nc.sync.dma_start(wenc_sb[d][:], wencT_d[sl, :])
                nc.sync.dma_start(enc_sb[d][:], encT_d[sl, :])
            nc.sync.dma_start(ind_sb[:], ind_d[:])
            for jh in range(2):
                nc.sync.dma_start(wq_sb[jh][:], wq_d[jh])

            # unused comb rows (100+nt .. 128 of the tail chunk) must be
            # finite: they are multiplied by ind zeros.  96 is the nearest
            # legal partition start; rows 96..100+nt are overwritten later.
            lk = len(chunks) - 1
            nc.vector.memset(comb[96:128, lk * J:(lk + 1) * J], 0.0)

            def comb_sl(k, j):
                return comb[:, (k * NJ + j) * 128:(k * NJ + j + 1) * 128]

            # ---- transposed projections -----------------------------------
            # dpT(+benc): psum[u, j] = sum_d dec[d,u]*wdec[d,j] + benc[j]
            pp0 = pspool.tile([128, 2, 512], f32, tag="ps", name="ps")
            for d in range(ND):
                nc.tensor.matmul(
                    pp0[0:U, 0, :], dec_sb[d][:, 0:U], wdec_sb[d][:],
                    start=(d == 0), stop=False,
                )
            nc.tensor.matmul(
                pp0[0:U, 0, :], ones_sb[0:1, 0:U], bencr_sb[:],
                start=False, stop=True,
            )
            nc.scalar.activation(dpTb[0:U, :], pp0[0:U, 0, :],
                                 AF.Identity, scale=-S)

            # epT [200, J] produced once (full-width matmuls), staged fp16;
            # per-chunk rows are DMA'd into comb (engine writes can't start
            # at partition 100)
            esA = cpool.tile([128, J], f16, tag="esA", name="esA")
            esB = cpool.tile([128, J], f16, tag="esB", name="esB")
            ppe = pspool.tile([128, 2, 512], f32, tag="ps", name="ps")
            for d in range(ND):
                nc.tensor.matmul(ppe[:, 0, :], enc_sb[d][:, 0:128],
                                 wenc_sb[d][:], start=(d == 0), stop=(d == ND - 1))
            for d in range(ND):
                nc.tensor.matmul(ppe[0:T - 128, 1, :], enc_sb[d][:, 128:T],
                                 wenc_sb[d][:], start=(d == 0), stop=(d == ND - 1))
            nc.scalar.activation(esA[:], ppe[:, 0, :], AF.Identity, scale=-S)
            nc.scalar.activation(esB[0:T - 128, :], ppe[0:T - 128, 1, :],
                                 AF.Identity, scale=-S)

            # ---- software-pipelined main loop -----------------------------
            g_cur = {}
            CMAX = NT * U

            def build_comb(k):
                # stitch comb[k, j] tiles: epT rows via SBUF->SBUF DMA from
                # the staged esA/esB, dpT rows via DVE copies
                t0, nt = chunks[k]
                if t0 + nt <= 128:
                    nc.sync.dma_start(
                        comb[100:100 + nt, k * J:(k + 1) * J], esA[t0:t0 + nt, :])
                elif t0 >= 128:
                    nc.sync.dma_start(
                        comb[100:100 + nt, k * J:(k + 1) * J],
                        esB[t0 - 128:t0 - 128 + nt, :])
                else:
                    a = 128 - t0
                    nc.sync.dma_start(
                        comb[100:100 + a, k * J:(k + 1) * J], esA[t0:128, :])
                    nc.sync.dma_start(
                        comb[100 + a:100 + nt, k * J:(k + 1) * J],
                        esB[0:nt - a, :])
                for j in range(NJ):
                    nc.vector.tensor_copy(
                        comb_sl(k, j)[0:U, :], dpTb[0:U, j * 128:(j + 1) * 128])

            def produce(k, j):
                # pre psum slices -> tanh -> STT -> g fp8
                t0, nt = chunks[k]
                cols = nt * U
                jh, i = j >> 1, j & 1
                if i == 0:
                    g_cur[(k % 2, jh)] = gpool.tile(
                        [128, 2, CMAX], f8, tag=f"g{jh}", name=f"g{jh}")
                gt = g_cur[(k % 2, jh)]
                pend = []

                def flush_slice():
                    ppf, c0, w = pend.pop(0)
                    t1 = t1pool.tile([128, PSL], f16, tag="t1", name="t1")
                    nc.scalar.activation(t1[:, 0:w], ppf[:, 0:w], AF.Tanh,
                                         scale=float(-1.0 / S))
                    nc.vector.scalar_tensor_tensor(
                        gt[:, i, c0:c0 + w], t1[:, 0:w], SG, ppf[:, 0:w],
                        ALU.mult, ALU.add)

                for c0 in range(0, cols, PSL):
                    w = min(PSL, cols - c0)
                    pp = pspool.tile([128, 2, 512], f32, tag="ps", name="ps")
                    for half in range(0, w, 512):
                        n = min(512, w - half)
                        nc.tensor.matmul(
                            pp[:, half // 512, 0:n],
                            comb_sl(k, j), ind_sb[:, c0 + half:c0 + half + n],
                            start=True, stop=True,
                        )
                    pend.append((pp[:].rearrange("p a b -> p (a b)"), c0, w))
                    if len(pend) > 2:
                        flush_slice()
                while pend:
                    flush_slice()

            # per-chunk work units: ("tile", c, w)
            def units_of(k):
                t0, nt = chunks[k]
                cols = nt * U
                return [("tile", c, min(128, cols - c))
                        for c in range(0, cols, 128)]

            st_state = {"tile": None, "fill": 0, "r0": None}
            copy_ctr = [0]
            dve_ctr = [0.0]

            def flush_st():
                st, fill, r0 = st_state["tile"], st_state["fill"], st_state["r0"]
                if st is None or fill == 0:
                    return
                dst = out_d[r0:r0 + fill * 128, :].rearrange(
                    "(g p) v -> p g v", p=128)
                nc.sync.dma_start(dst, st[:, 0:fill, :])
                st_state["tile"] = None
                st_state["fill"] = 0
                st_state["r0"] = None

            def copy_out(src_ap, dst_ap):
                dve_ctr[0] += DVE_COPY_FRAC
                if dve_ctr[0] >= 1.0:
                    dve_ctr[0] -= 1.0
                    nc.vector.tensor_scalar_mul(dst_ap, src_ap, OUT_SCALE)
                else:
                    nc.scalar.activation(dst_ap, src_ap, AF.Copy, scale=OUT_SCALE)

            pend_units = []

            def evac_unit():
                k, ps, c, w = pend_units.pop(0)
                t0, nt = chunks[k]
                tu0 = t0 * U
                if w == 128:
                    if st_state["tile"] is None:
                        st_state["tile"] = stpool.tile([128, GRP, V], f16,
                                                       tag="st", name="st")
                        st_state["r0"] = tu0 + c
                    st = st_state["tile"]
                    s0 = st_state["fill"]
                    copy_out(ps[:].rearrange("p a b -> p (a b)"),
                             st[:, s0, :])
                    st_state["fill"] += 1
                    if st_state["fill"] == GRP:
                        flush_st()
                else:
                    flush_st()
                    stp = stppool.tile([128, 1, V], f16, tag="stp", name="stp")
                    copy_out(ps[0:w, :, :].rearrange("p a b -> p (a b)"),
                             stp[0:w, 0, :])
                    nc.sync.dma_start(out_d[tu0 + c:tu0 + c + w, :],
                                      stp[0:w, 0, :])

            def process_unit(k, unit):
                _, c, w = unit
                ps = pspool.tile([128, 2, 512], f32, tag="ps", name="ps")
                for jh in range(2):
                    for h in range(2):
                        nc.tensor.matmul(
                            ps[0:w, h, :],
                            g_cur[(k % 2, jh)][:, :, c:c + w],
                            wq_sb[jh][:, :, h * 512:(h + 1) * 512],
                            start=(jh == 0), stop=(jh == 1),
                            perf_mode=DR,
                        )
                pend_units.append((k, ps, c, w))
                if len(pend_units) > 1:
                    evac_unit()

            # pipeline: chunk k's g production interleaved with chunk k-1's
            # matmuls + copies.  build_comb(k+1) runs one chunk ahead.
            build_comb(0)
            for k in range(NCH + 1):
                units = units_of(k - 1) if k > 0 else []
                nu = len(units)
                bounds = [nu * i // 5 for i in range(6)]
                for j in range(4):
                    if k < NCH:
                        if j == 2 and k + 1 < NCH:
                            build_comb(k + 1)
                        produce(k, j)
                    for unit in units[bounds[j]:bounds[j + 1]]:
                        process_unit(k - 1, unit)
                for unit in units[bounds[4]:bounds[5]]:
                    process_unit(k - 1, unit)
            while pend_units:
                evac_unit()
            flush_st()

    _fix_matmul_waits(nc)
    return nc


def _fix_matmul_waits(nc):
    """TRN2 TPB instructions take at most 1 semaphore wait (EventSemaphore: 2),
    but Tile emits up to 4 on one instruction. For each saturated compute
    instruction, park the excess waits on EventSemaphore instructions inserted
    immediately before it on the same engine (no reordering, so the schedule's
    correctness argument is untouched)."""
    import concourse.mybir as mybir

    capped = (
        mybir.InstMatmult, mybir.InstLdweights, mybir.InstActivation,
        mybir.InstTensorTensor, mybir.InstTensorCopy, mybir.InstMemset,
        mybir.InstTensorReduce, mybir.InstDMACopy, mybir.InstDrain,
        mybir.InstTensorScalarPtr,
    )
    _n = [0]
    for f in nc.m.functions:
        for blk in f.blocks:
            fixups = []
            for inst in blk.instructions:
                if not isinstance(inst, capped):
                    continue
                si = inst.sync_info
                if si is None or len(si.on_wait) <= 1:
                    continue
                waits = list(si.on_wait)
                fixups.append((inst, waits[:-1]))
                si.on_wait = waits[-1:]
            for inst, excess in fixups:
                idx = blk.instructions.index(inst)
                for i in range(0, len(excess), 2):
                    ev = mybir.InstEventSemaphore(
                        name=f"waitfix-{_n[0]}",
                        engine=inst.engine,
                        sync_info=mybir.SyncInfo(
                            on_wait=excess[i:i + 2], on_update=[]),
                    )
                    _n[0] += 1
                    blk.instructions.insert(idx, ev)
                    idx += 1


def _get_nc():
    if "nc" not in _CACHE:
        _CACHE["nc"] = _build()
    return _CACHE["nc"]


def _make_ind():
    ind = np.zeros((128, NT * U), np.float32)
    n = np.arange(NT * U)
    ind[n % U, n] = 1.0          # u rows
    ind[U + n // U, n] = 1.0     # t rows
    return ind.astype(np.float16)


def _prep(inputs):
    import ml_dtypes

    enc_out = np.asarray(inputs["enc_out"], np.float32)   # (B,T,1,D)
    dec_out = np.asarray(inputs["dec_out"], np.float32)   # (B,1,U,D)
    W_enc = np.asarray(inputs["W_enc"], np.float32)       # (J,D)
    W_dec = np.asarray(inputs["W_dec"], np.float32)       # (J,D)
    W_out = np.asarray(inputs["W_out"], np.float32)       # (V,J)
    b_enc = np.asarray(inputs["b_enc"], np.float32)       # (J,)

    encT = np.ascontiguousarray(enc_out[:, :, 0, :].transpose(0, 2, 1))  # (B,D,T)
    decT = np.ascontiguousarray(dec_out[:, 0, :, :].transpose(0, 2, 1))  # (B,D,U)
    wencT = np.ascontiguousarray(W_enc.T)                                # (D,J)
    wdecT = np.ascontiguousarray(W_dec.T)                                # (D,J)
    bencr = np.ascontiguousarray(b_enc.reshape(1, J))
    ones = np.ones((1, 128), np.float32)
    ind = _make_ind()

    # W_out fp8: wq[jh, p, i, v] = e4m3(W_out[v, (2jh+i)*128+p] * SW)
    wt = np.clip(W_out.T * SW, -240.0, 240.0)             # (J, V)
    wq = np.ascontiguousarray(
        wt.reshape(2, 2, 128, V).transpose(0, 2, 1, 3)
    ).astype(ml_dtypes.float8_e4m3)                       # (2,128,2,V)

    in_maps = [
        dict(encT=encT[b], decT=decT[b], wencT=wencT, wdecT=wdecT,
             bencr=bencr, ones=ones, ind=ind, wq=wq)
        for b in range(B)
    ]

    # host-side linear part: alpha*(W@(e+benc) (+) W@d) + b_out
    ep = np.einsum('bdt,dj->btj', encT, wencT) + b_enc    # (B,T,J) f32
    dpj = np.einsum('bdu,dj->buj', decT, wdecT)           # (B,U,J)
    LT = ALPHA * np.einsum('btj,vj->btv', ep, W_out)      # (B,T,V)
    LU = ALPHA * np.einsum('buj,vj->buv', dpj, W_out)     # (B,U,V)
    LU = LU + np.asarray(inputs["b_out"], np.float32)[None, None, :]
    return in_maps, LT, LU


def _run(inputs, trace=False):
    from concourse.bass_utils import run_bass_kernel_spmd

    in_maps, LT, LU = _prep(inputs)
    nc = _get_nc()
    res = run_bass_kernel_spmd(nc, in_maps, list(range(B)), trace=trace)
    outs = np.stack([np.asarray(res.results[i]["out"]) for i in range(B)])
    out = outs.reshape(B, T, U, V).astype(np.float32)
    out += LT[:, :, None, :]
    out += LU[:, None, :, :]
    return np.ascontiguousarray(out, dtype=np.float32), res


def kernel(**inputs):
    out, _ = _run(inputs)
    return out

